# revision 14
# baseline (speedup 1.0000x reference)
"""CLIP contrastive loss (nn_ClipLoss) on 8 Trainium2 NeuronCores.

Strategy (row-sharded data parallel, fp8 DoubleRow matmuls):
  - Each core k gets its row shard of image embeddings (bf16 natural, for
    on-device norms + diag) plus host-transposed fp8-e4m3 matmul operands:
    imgT8 [D, m_loc] (raw) and the replicated txtT8 [D, N] with the text
    normalization 16/||b_j|| pre-folded (the sharding hint's "all-gather the
    other modality's normalized embeddings", minus the gather: text norms are
    an O(N*D) input-prep step, 0.006% of the N^2*D device FLOPs; f32 host
    norms are more precise than the on-device bf16 path they replace).
  - The big logits matmul runs in fp8 with MatmulPerfMode.DoubleRow: operand
    tiles are [128, 2, F] (dim1 = two k-subtiles), K=256 per instruction at
    0.5 cycles/output-row -> 4x the bf16 matmul throughput. Raw image rows
    are ~N(0,1) and scaled text rows ~N(0,0.25), both in e4m3's sweet spot;
    quantization adds ~1e-3 relative loss error.
  - Image-side 1/(16*T*||a_i||) rides the ACT exp `scale` per-partition
    operand, so PSUM needs no elementwise fixups at all: exp reads the raw
    accumulated logits. exp ops span a PAIR of column groups ([128, 2048])
    to halve the per-op ACT overhead; fused accum_out yields row sums.
  - Column sums accumulate on DVE (bf16 2x mode), are partition-reduced by
    ones-matmuls into a stolen mm-tag PSUM rotation slot, staged bf16, and
    summed across cores with a 16KB ReduceScatter (each core gets exactly
    its own column shard -> no per-core addressing).
  - Each core emits one fp32 partial; host sums 8 partials.
"""

import math

import numpy as np
import ml_dtypes

N_FULL = 8192
D_FULL = 1024
W = 8
P = 128
NSLICE = 512
GPW = 2048                     # fused column-group pair width
TEMP = 0.07
BSCALE = 16.0                  # fp8 range scaling folded into txtT8
LN_INV_ST = math.log(1.0 / (BSCALE * TEMP))

_CACHE: dict = {}


def build_bass(n_global: int = N_FULL, d: int = D_FULL, collectives: bool = True):
    """Build the SPMD bass program (identical on all cores).

    collectives=False replaces the ReduceScatter with a local DMA stand-in
    (for single-core TimelineSim cost modeling only — numerically wrong
    across cores, but dependency/traffic equivalent on one core).
    """
    from contextlib import ExitStack

    import concourse.mybir as mybir
    import concourse.tile as tile
    from concourse import bacc

    f32 = mybir.dt.float32
    bf16 = mybir.dt.bfloat16
    fp8 = mybir.dt.float8e4
    AF = mybir.ActivationFunctionType
    OP = mybir.AluOpType
    X = mybir.AxisListType.X
    DR = mybir.MatmulPerfMode.DoubleRow

    m_loc = n_global // W          # rows per core
    mc_n = m_loc // P              # row chunks per core
    cp_n = d // (2 * P)            # DoubleRow contraction pair-chunks
    q_n = n_global // GPW          # fused group pairs
    ns_n = GPW // NSLICE           # 512-wide matmul slices per pair
    assert n_global % GPW == 0 and GPW % NSLICE == 0

    import concourse.bacc as bacc_mod

    if not getattr(bacc_mod, "_clip_act_tables_patched", False):
        _orig_tabs = bacc_mod.get_activation_tables

        def _one_set_tables(module_arch):
            tabs = dict(_orig_tabs(module_arch))
            full_name = "natural_log_exp_and_others"
            if full_name in tabs:
                ours = {AF.Ln, AF.Exp, AF.Copy, AF.Identity, AF.Square}
                for name in tabs:
                    if name != full_name:
                        tabs[name] = set(tabs[name]) - ours
            return tabs

        bacc_mod.get_activation_tables = _one_set_tables
        bacc_mod._clip_act_tables_patched = True

    nc = bacc.Bacc("TRN2", target_bir_lowering=False, num_devices=W)
    img8 = nc.dram_tensor("img8", [m_loc, d], fp8, kind="ExternalInput")
    txtn8 = nc.dram_tensor("txtn8", [m_loc, d], fp8, kind="ExternalInput")
    # fp8 transposed operands, pre-grouped for DoubleRow: [cp, 2, 128, cols]
    txtT8 = nc.dram_tensor("txtT8", [cp_n, 2, P, n_global], fp8,
                           kind="ExternalInput")
    imgT8 = nc.dram_tensor("imgT8", [cp_n, 2, P, m_loc], fp8,
                           kind="ExternalInput")
    out_d = nc.dram_tensor("partial", [1, 1], f32, kind="ExternalOutput")
    rg = [list(range(W))]

    with tile.TileContext(nc) as tc, ExitStack() as ctx:
        sb = ctx.enter_context(tc.tile_pool(name="sb", bufs=1))
        ps = ctx.enter_context(tc.tile_pool(name="ps", bufs=1, space="PSUM"))
        dram = ctx.enter_context(tc.tile_pool(name="dram", bufs=1, space="DRAM"))

        # constants
        ones_bf = sb.tile([P, 1], bf16, name="ones_bf")
        nc.gpsimd.memset(ones_bf[:], 1.0)
        ones_f32 = sb.tile([P, 1], f32, name="ones_f32")
        nc.gpsimd.memset(ones_f32[:], 1.0)
        ln_invst = sb.tile([P, 1], f32, name="ln_invst")
        nc.gpsimd.memset(ln_invst[:], LN_INV_ST)

        # collective DRAM buffers (column-sum ReduceScatter, bf16)
        cc_rs_in = dram.tile([1, n_global], bf16, name="cc_rs_in")
        cc_rs_out = dram.tile([1, m_loc], bf16, name="cc_rs_out")

        # ---------------- prologue ------------------------------------------
        # HWDGE queues serialize at ~1.3us fixed cost per DMA, but the gpsimd
        # SWDGE path sprays descriptors across DMA engines in parallel: the
        # fp8 matmul streams all ride gpsimd. The bf16 naturals are batched
        # into 3 big strided DMAs on the scalar HWDGE queue (a-halves first:
        # they gate the norms -> ra_act -> first-exp chain).
        half = mc_n // 2

        def load_bT(q):
            t = sb.tile([P, cp_n, 2, GPW], fp8, name="bT", tag="bT", bufs=3)
            nc.gpsimd.dma_start(
                t[:],
                txtT8[:, :, :, q * GPW:(q + 1) * GPW].rearrange(
                    "cp two p n -> p cp two n"
                ),
            )
            return t

        aT_t = sb.tile([P, cp_n, 2, m_loc], fp8, name="aT")
        nc.gpsimd.dma_start(
            aT_t[:], imgT8[:].rearrange("cp two p n -> p cp two n")
        )
        a_big = sb.tile([P, mc_n, d], fp8, name="a_big")
        nc.scalar.dma_start(
            a_big[:], img8[:, :].rearrange("(m p) k -> p m k", p=P)
        )
        a_nats = [a_big[:, m, :] for m in range(mc_n)]
        bT_pre = {0: load_bT(0), 1: load_bT(1)}
        b_big = sb.tile([P, mc_n, d], fp8, name="b_big")
        b_nats = [b_big[:, m, :] for m in range(mc_n)]

        norms2_a = sb.tile([P, mc_n], f32, name="norms2_a")
        d_nat = sb.tile([P, mc_n], f32, name="d_nat")

        # a-side norms -> image exp scale ra = 1/(16*T*||a_i||), computed in
        # two chunks so the first exp only waits on the first two row blocks
        ln_a = sb.tile([P, mc_n], f32, name="ln_a")
        ra_act = sb.tile([P, mc_n], f32, name="ra_act")
        for lo, hi in ((0, half), (half, mc_n)):
            for m in range(lo, hi):
                sqa = sb.tile([P, d], bf16, name="sqa", tag="sqa", bufs=2)
                nc.vector.scalar_tensor_tensor(
                    out=sqa[:], in0=a_nats[m], scalar=1.0, in1=a_nats[m],
                    op0=OP.mult, op1=OP.mult, accum_out=norms2_a[:, m:m + 1],
                )
            nc.scalar.activation(ln_a[:, lo:hi], norms2_a[:, lo:hi], AF.Ln)
            nc.scalar.activation(ra_act[:, lo:hi], ln_a[:, lo:hi], AF.Exp,
                                 scale=-0.5, bias=ln_invst[:])

        # row-sum accumulator: slot (q*mc_n + m) <- sum_j exp over group pair
        row_acc = sb.tile([P, q_n * mc_n], f32, name="row_acc")

        # ---------------- main loop over column group pairs ------------------
        col_accs = []
        for q in range(q_n):
            bTq = bT_pre[q] if q in bT_pre else load_bT(q)
            if q == 2:
                # b naturals (diag-only) load late: keeps the DMA device
                # clear for the exp-gating streams
                nc.scalar.dma_start(
                    b_big[:],
                    txtn8[:, :].rearrange("(m p) k -> p m k", p=P),
                )

            col_acc = sb.tile([P, GPW], bf16, name="col_acc", tag="col_acc",
                              bufs=q_n)
            col_accs.append(col_acc)
            for m in range(mc_n):
                mm_ps = ps.tile([P, GPW], f32, name="mm_ps", tag="mm", bufs=2)
                for n2 in range(ns_n):
                    for c in range(cp_n):
                        nc.tensor.matmul(
                            mm_ps[:, n2 * NSLICE:(n2 + 1) * NSLICE],
                            aT_t[:, c, :, m * P:(m + 1) * P],
                            bTq[:, c, :, n2 * NSLICE:(n2 + 1) * NSLICE],
                            start=(c == 0),
                            stop=(c == cp_n - 1),
                            perf_mode=DR,
                        )
                exp_t = sb.tile([P, GPW], bf16, name="exp_t", tag="exp_t",
                                bufs=3)
                slot = q * mc_n + m
                nc.scalar.activation(
                    exp_t[:],
                    mm_ps[:],
                    AF.Exp,
                    scale=ra_act[:, m:m + 1],
                    accum_out=row_acc[:, slot:slot + 1],
                )
                if m == 0:
                    nc.vector.tensor_copy(col_acc[:], exp_t[:])
                else:
                    nc.vector.tensor_add(col_acc[:], col_acc[:], exp_t[:])

            if q == 2:
                # diag terms: host-normalized b_nat carries 16/||b_i||, so
                # dterm = d_nat * ra_act == (a_i.b_i)/(T ||a_i|| ||b_i||)
                # exactly; DVE has slack mid-loop.
                for m in range(mc_n):
                    prod = sb.tile([P, d], bf16, name="prod", tag="prod",
                                   bufs=2)
                    nc.vector.scalar_tensor_tensor(
                        out=prod[:], in0=a_nats[m], scalar=1.0,
                        in1=b_nats[m],
                        op0=OP.mult, op1=OP.mult,
                        accum_out=d_nat[:, m:m + 1],
                    )

        dterm = sb.tile([P, mc_n], f32, name="dterm")
        nc.vector.tensor_mul(dterm[:], d_nat[:], ra_act[:])

        # ---- deferred column-sum partition reduction (tail): ones-matmuls
        # into stolen mm-tag PSUM slots, staged bf16 for the ReduceScatter.
        # Copies alternate DVE / idle ACT.
        for q in range(q_n):
            cs_ps = ps.tile([P, GPW], f32, name="cs_ps", tag="mm", bufs=2)
            for n2 in range(ns_n):
                nc.tensor.matmul(
                    cs_ps[0:1, n2 * NSLICE:(n2 + 1) * NSLICE], ones_bf[:],
                    col_accs[q][:, n2 * NSLICE:(n2 + 1) * NSLICE],
                    start=True, stop=True,
                )
            cs_row = sb.tile([1, GPW], bf16, name="cs_row", tag="cs_row",
                             bufs=2)
            if q % 2 == 0:
                nc.vector.tensor_copy(cs_row[:], cs_ps[0:1, :])
            else:
                nc.scalar.activation(cs_row[:], cs_ps[0:1, :], AF.Copy)
            nc.gpsimd.dma_start(cc_rs_in[0:1, q * GPW:(q + 1) * GPW],
                                cs_row[:])

        # ---------------- epilogue -------------------------------------------
        if collectives:
            nc.gpsimd.collective_compute(
                "ReduceScatter",
                OP.add,
                replica_groups=rg,
                ins=[cc_rs_in[:].opt()],
                outs=[cc_rs_out[:].opt()],
            )
        else:
            nc.gpsimd.dma_start(cc_rs_out[:], cc_rs_in[0:1, 0:m_loc])

        # my column shard's summed exp: [P, mc_n] (element order irrelevant)
        scol = sb.tile([P, mc_n], bf16, name="scol")
        nc.gpsimd.dma_start(
            scol[:], cc_rs_out[0:1, :].rearrange("o (p f) -> p (o f)", p=P)
        )
        lsc = sb.tile([P, mc_n], f32, name="lsc")
        nc.scalar.activation(lsc[:], scol[:], AF.Ln)

        # total row sums: sum slots over q for each m
        srow = sb.tile([P, mc_n], f32, name="srow")
        nc.vector.tensor_reduce(
            srow[:],
            row_acc[:].rearrange("p (q m) -> p m q", q=q_n),
            axis=X,
            op=OP.add,
        )
        lsr = sb.tile([P, mc_n], f32, name="lsr")
        nc.scalar.activation(lsr[:], srow[:], AF.Ln)

        # per-partition combine: F = 0.5*(sum lsr + sum lsc) - sum dterm
        s1 = sb.tile([P, 1], f32, name="s1")
        nc.vector.tensor_reduce(s1[:], lsr[:], axis=X, op=OP.add)
        s2 = sb.tile([P, 1], f32, name="s2")
        nc.vector.tensor_reduce(s2[:], lsc[:], axis=X, op=OP.add)
        s3 = sb.tile([P, 1], f32, name="s3")
        nc.vector.tensor_reduce(s3[:], dterm[:], axis=X, op=OP.add)
        tsum = sb.tile([P, 1], f32, name="tsum")
        nc.vector.tensor_add(tsum[:], s1[:], s2[:])
        fvec = sb.tile([P, 1], f32, name="fvec")
        nc.vector.scalar_tensor_tensor(
            out=fvec[:], in0=tsum[:], scalar=0.5, in1=s3[:],
            op0=OP.mult, op1=OP.subtract,
        )

        # partition sum -> scalar partial (scaled by 1/N), in a stolen
        # mm-tag PSUM slot
        loss_ps = ps.tile([P, GPW], f32, name="loss_ps", tag="mm", bufs=2)
        nc.tensor.matmul(loss_ps[0:1, 0:1], ones_f32[:], fvec[:], start=True,
                         stop=True)
        out_sb = sb.tile([1, 1], f32, name="out_sb")
        nc.scalar.mul(out_sb[:], loss_ps[0:1, 0:1], 1.0 / n_global)
        nc.gpsimd.dma_start(out_d[0:1, 0:1], out_sb[:])

    nc.compile()
    return nc


def make_in_maps(image_embeddings: np.ndarray, text_embeddings: np.ndarray):
    n_global, d = image_embeddings.shape
    m_loc = n_global // W
    cp_n = d // (2 * P)
    # host text normalization (f32): fold 16/||b_j|| into both text operands
    bn = np.sqrt((text_embeddings.astype(np.float32) ** 2).sum(axis=1,
                                                               keepdims=True))
    txt_n = text_embeddings * (BSCALE / np.maximum(bn, 1e-12))
    img8 = image_embeddings.astype(ml_dtypes.float8_e4m3fn)
    txtn8 = txt_n.astype(ml_dtypes.float8_e4m3fn)
    # fp8 transposed operands grouped for DoubleRow: [cp, 2, 128, cols]
    txtT8 = np.ascontiguousarray(
        np.ascontiguousarray(txtn8.T).reshape(cp_n, 2, P, n_global)
    )
    imgT8_full = np.ascontiguousarray(img8.T).reshape(cp_n, 2, P, n_global)
    return [
        {
            "img8": img8[k * m_loc:(k + 1) * m_loc],
            "txtn8": txtn8[k * m_loc:(k + 1) * m_loc],
            "txtT8": txtT8,
            "imgT8": np.ascontiguousarray(
                imgT8_full[:, :, :, k * m_loc:(k + 1) * m_loc]
            ),
        }
        for k in range(W)
    ]


def kernel(image_embeddings: np.ndarray, text_embeddings: np.ndarray) -> np.ndarray:
    from concourse.bass_utils import run_bass_kernel_spmd

    n_global, d = image_embeddings.shape
    key = (n_global, d)
    if key not in _CACHE:
        _CACHE[key] = build_bass(n_global, d)
    nc = _CACHE[key]

    in_maps = make_in_maps(
        np.asarray(image_embeddings, np.float32),
        np.asarray(text_embeddings, np.float32),
    )
    res = run_bass_kernel_spmd(nc, in_maps, core_ids=list(range(W)))
    total = sum(float(r["partial"][0, 0]) for r in res.results)
    return np.asarray(total, dtype=np.float32)


# revision 15
# speedup vs baseline: 1.0453x; 1.0453x over previous
"""CLIP contrastive loss (nn_ClipLoss) on 8 Trainium2 NeuronCores.

Strategy (row-sharded data parallel, fp8 DoubleRow matmuls):
  - Each core k gets its row shard of image embeddings (bf16 natural, for
    on-device norms + diag) plus host-transposed fp8-e4m3 matmul operands:
    imgT8 [D, m_loc] (raw) and the replicated txtT8 [D, N] with the text
    normalization 16/||b_j|| pre-folded (the sharding hint's "all-gather the
    other modality's normalized embeddings", minus the gather: text norms are
    an O(N*D) input-prep step, 0.006% of the N^2*D device FLOPs; f32 host
    norms are more precise than the on-device bf16 path they replace).
  - The big logits matmul runs in fp8 with MatmulPerfMode.DoubleRow: operand
    tiles are [128, 2, F] (dim1 = two k-subtiles), K=256 per instruction at
    0.5 cycles/output-row -> 4x the bf16 matmul throughput. Raw image rows
    are ~N(0,1) and scaled text rows ~N(0,0.25), both in e4m3's sweet spot;
    quantization adds ~1e-3 relative loss error.
  - Image-side 1/(16*T*||a_i||) rides the ACT exp `scale` per-partition
    operand, so PSUM needs no elementwise fixups at all: exp reads the raw
    accumulated logits. exp ops span a PAIR of column groups ([128, 2048])
    to halve the per-op ACT overhead; fused accum_out yields row sums.
  - Column sums accumulate on DVE (bf16 2x mode), are partition-reduced by
    ones-matmuls into a stolen mm-tag PSUM rotation slot, staged bf16, and
    summed across cores with a 16KB ReduceScatter (each core gets exactly
    its own column shard -> no per-core addressing).
  - Each core emits one fp32 partial; host sums 8 partials.
"""

import math

import numpy as np
import ml_dtypes

N_FULL = 8192
D_FULL = 1024
W = 8
P = 128
NSLICE = 512
GPW = 2048                     # fused column-group pair width
TEMP = 0.07
BSCALE = 16.0                  # fp8 range scaling folded into txtT8
LN_INV_ST = math.log(1.0 / (BSCALE * TEMP))

_CACHE: dict = {}


def build_bass(n_global: int = N_FULL, d: int = D_FULL, collectives: bool = True):
    """Build the SPMD bass program (identical on all cores).

    collectives=False replaces the ReduceScatter with a local DMA stand-in
    (for single-core TimelineSim cost modeling only — numerically wrong
    across cores, but dependency/traffic equivalent on one core).
    """
    from contextlib import ExitStack

    import concourse.mybir as mybir
    import concourse.tile as tile
    from concourse import bacc

    f32 = mybir.dt.float32
    bf16 = mybir.dt.bfloat16
    fp8 = mybir.dt.float8e4
    AF = mybir.ActivationFunctionType
    OP = mybir.AluOpType
    X = mybir.AxisListType.X
    DR = mybir.MatmulPerfMode.DoubleRow

    m_loc = n_global // W          # rows per core
    mc_n = m_loc // P              # row chunks per core
    cp_n = d // (2 * P)            # DoubleRow contraction pair-chunks
    q_n = n_global // GPW          # fused group pairs
    ns_n = GPW // NSLICE           # 512-wide matmul slices per pair
    assert n_global % GPW == 0 and GPW % NSLICE == 0

    import concourse.bacc as bacc_mod

    if not getattr(bacc_mod, "_clip_act_tables_patched", False):
        _orig_tabs = bacc_mod.get_activation_tables

        def _one_set_tables(module_arch):
            tabs = dict(_orig_tabs(module_arch))
            full_name = "natural_log_exp_and_others"
            if full_name in tabs:
                ours = {AF.Ln, AF.Exp, AF.Copy, AF.Identity, AF.Square}
                for name in tabs:
                    if name != full_name:
                        tabs[name] = set(tabs[name]) - ours
            return tabs

        bacc_mod.get_activation_tables = _one_set_tables
        bacc_mod._clip_act_tables_patched = True

    nc = bacc.Bacc("TRN2", target_bir_lowering=False, num_devices=W)
    img8 = nc.dram_tensor("img8", [m_loc, d], fp8, kind="ExternalInput")
    txtn8 = nc.dram_tensor("txtn8", [m_loc, d], fp8, kind="ExternalInput")
    # fp8 transposed operands, pre-grouped for DoubleRow: [cp, 2, 128, cols]
    txtT8 = nc.dram_tensor("txtT8", [cp_n, 2, P, n_global], fp8,
                           kind="ExternalInput")
    imgT8 = nc.dram_tensor("imgT8", [cp_n, 2, P, m_loc], fp8,
                           kind="ExternalInput")
    out_d = nc.dram_tensor("partial", [1, 1], f32, kind="ExternalOutput")
    rg = [list(range(W))]

    with tile.TileContext(nc) as tc, ExitStack() as ctx:
        sb = ctx.enter_context(tc.tile_pool(name="sb", bufs=1))
        ps = ctx.enter_context(tc.tile_pool(name="ps", bufs=1, space="PSUM"))
        dram = ctx.enter_context(tc.tile_pool(name="dram", bufs=1, space="DRAM"))

        # constants
        ones_bf = sb.tile([P, 1], bf16, name="ones_bf")
        nc.gpsimd.memset(ones_bf[:], 1.0)
        ones_f32 = sb.tile([P, 1], f32, name="ones_f32")
        nc.gpsimd.memset(ones_f32[:], 1.0)
        ln_invst = sb.tile([P, 1], f32, name="ln_invst")
        nc.gpsimd.memset(ln_invst[:], LN_INV_ST)

        # collective DRAM buffers (column-sum ReduceScatter, bf16)
        cc_rs_in = dram.tile([1, n_global], bf16, name="cc_rs_in")
        cc_rs_out = dram.tile([1, m_loc], bf16, name="cc_rs_out")

        # ---------------- prologue ------------------------------------------
        # HWDGE queues serialize at ~1.3us fixed cost per DMA, but the gpsimd
        # SWDGE path sprays descriptors across DMA engines in parallel: the
        # fp8 matmul streams all ride gpsimd. The bf16 naturals are batched
        # into 3 big strided DMAs on the scalar HWDGE queue (a-halves first:
        # they gate the norms -> ra_act -> first-exp chain).
        half = mc_n // 2

        def load_bT(q):
            tiles = []
            for c in range(cp_n):
                t = sb.tile([P, 2, GPW], fp8, name="bT", tag=f"bT{c}", bufs=3)
                nc.gpsimd.dma_start(
                    t[:],
                    txtT8[c, :, :, q * GPW:(q + 1) * GPW].rearrange(
                        "two p n -> p two n"
                    ),
                )
                tiles.append(t)
            return tiles

        aT = []
        for c in range(cp_n):
            t = sb.tile([P, 2, m_loc], fp8, name=f"aT{c}")
            nc.gpsimd.dma_start(t[:], imgT8[c].rearrange("two p n -> p two n"))
            aT.append(t)
        a_big = sb.tile([P, mc_n, d], fp8, name="a_big")
        nc.scalar.dma_start(
            a_big[:], img8[:, :].rearrange("(m p) k -> p m k", p=P)
        )
        a_nats = [a_big[:, m, :] for m in range(mc_n)]
        bT_pre = {0: load_bT(0), 1: load_bT(1)}
        b_big = sb.tile([P, mc_n, d], fp8, name="b_big")
        b_nats = [b_big[:, m, :] for m in range(mc_n)]

        norms2_a = sb.tile([P, mc_n], f32, name="norms2_a")
        d_nat = sb.tile([P, mc_n], f32, name="d_nat")

        # a-side norms -> image exp scale ra = 1/(16*T*||a_i||), computed in
        # two chunks so the first exp only waits on the first two row blocks
        ln_a = sb.tile([P, mc_n], f32, name="ln_a")
        ra_act = sb.tile([P, mc_n], f32, name="ra_act")
        for lo, hi in ((0, half), (half, mc_n)):
            for m in range(lo, hi):
                sqa = sb.tile([P, d], bf16, name="sqa", tag="sqa", bufs=2)
                nc.vector.scalar_tensor_tensor(
                    out=sqa[:], in0=a_nats[m], scalar=1.0, in1=a_nats[m],
                    op0=OP.mult, op1=OP.mult, accum_out=norms2_a[:, m:m + 1],
                )
            nc.scalar.activation(ln_a[:, lo:hi], norms2_a[:, lo:hi], AF.Ln)
            nc.scalar.activation(ra_act[:, lo:hi], ln_a[:, lo:hi], AF.Exp,
                                 scale=-0.5, bias=ln_invst[:])

        # row-sum accumulator: slot (q*mc_n + m) <- sum_j exp over group pair
        row_acc = sb.tile([P, q_n * mc_n], f32, name="row_acc")

        # ---------------- main loop over column group pairs ------------------
        col_accs = []
        for q in range(q_n):
            bTq = bT_pre[q] if q in bT_pre else load_bT(q)
            if q == 2:
                # b naturals (diag-only) load late: keeps the DMA device
                # clear for the exp-gating streams
                nc.scalar.dma_start(
                    b_big[:],
                    txtn8[:, :].rearrange("(m p) k -> p m k", p=P),
                )

            col_acc = sb.tile([P, GPW], bf16, name="col_acc", tag="col_acc",
                              bufs=q_n)
            col_accs.append(col_acc)
            for m in range(mc_n):
                mm_ps = ps.tile([P, GPW], f32, name="mm_ps", tag="mm", bufs=2)
                for n2 in range(ns_n):
                    for c in range(cp_n):
                        nc.tensor.matmul(
                            mm_ps[:, n2 * NSLICE:(n2 + 1) * NSLICE],
                            aT[c][:, :, m * P:(m + 1) * P],
                            bTq[c][:, :, n2 * NSLICE:(n2 + 1) * NSLICE],
                            start=(c == 0),
                            stop=(c == cp_n - 1),
                            perf_mode=DR,
                        )
                exp_t = sb.tile([P, GPW], bf16, name="exp_t", tag="exp_t",
                                bufs=3)
                slot = q * mc_n + m
                nc.scalar.activation(
                    exp_t[:],
                    mm_ps[:],
                    AF.Exp,
                    scale=ra_act[:, m:m + 1],
                    accum_out=row_acc[:, slot:slot + 1],
                )
                if m == 0:
                    nc.vector.tensor_copy(col_acc[:], exp_t[:])
                else:
                    nc.vector.tensor_add(col_acc[:], col_acc[:], exp_t[:])

            if q == 2:
                # diag terms: host-normalized b_nat carries 16/||b_i||, so
                # dterm = d_nat * ra_act == (a_i.b_i)/(T ||a_i|| ||b_i||)
                # exactly; DVE has slack mid-loop.
                for m in range(mc_n):
                    prod = sb.tile([P, d], bf16, name="prod", tag="prod",
                                   bufs=2)
                    nc.vector.scalar_tensor_tensor(
                        out=prod[:], in0=a_nats[m], scalar=1.0,
                        in1=b_nats[m],
                        op0=OP.mult, op1=OP.mult,
                        accum_out=d_nat[:, m:m + 1],
                    )

        dterm = sb.tile([P, mc_n], f32, name="dterm")
        nc.vector.tensor_mul(dterm[:], d_nat[:], ra_act[:])

        # ---- deferred column-sum partition reduction (tail): ones-matmuls
        # into stolen mm-tag PSUM slots, staged bf16 for the ReduceScatter.
        # Copies alternate DVE / idle ACT.
        for q in range(q_n):
            cs_ps = ps.tile([P, GPW], f32, name="cs_ps", tag="mm", bufs=2)
            for n2 in range(ns_n):
                nc.tensor.matmul(
                    cs_ps[0:1, n2 * NSLICE:(n2 + 1) * NSLICE], ones_bf[:],
                    col_accs[q][:, n2 * NSLICE:(n2 + 1) * NSLICE],
                    start=True, stop=True,
                )
            cs_row = sb.tile([1, GPW], bf16, name="cs_row", tag="cs_row",
                             bufs=2)
            if q % 2 == 0:
                nc.vector.tensor_copy(cs_row[:], cs_ps[0:1, :])
            else:
                nc.scalar.activation(cs_row[:], cs_ps[0:1, :], AF.Copy)
            nc.gpsimd.dma_start(cc_rs_in[0:1, q * GPW:(q + 1) * GPW],
                                cs_row[:])

        # ---------------- epilogue -------------------------------------------
        if collectives:
            nc.gpsimd.collective_compute(
                "ReduceScatter",
                OP.add,
                replica_groups=rg,
                ins=[cc_rs_in[:].opt()],
                outs=[cc_rs_out[:].opt()],
            )
        else:
            nc.gpsimd.dma_start(cc_rs_out[:], cc_rs_in[0:1, 0:m_loc])

        # my column shard's summed exp: [P, mc_n] (element order irrelevant)
        scol = sb.tile([P, mc_n], bf16, name="scol")
        nc.gpsimd.dma_start(
            scol[:], cc_rs_out[0:1, :].rearrange("o (p f) -> p (o f)", p=P)
        )
        lsc = sb.tile([P, mc_n], f32, name="lsc")
        nc.scalar.activation(lsc[:], scol[:], AF.Ln)

        # total row sums: sum slots over q for each m
        srow = sb.tile([P, mc_n], f32, name="srow")
        nc.vector.tensor_reduce(
            srow[:],
            row_acc[:].rearrange("p (q m) -> p m q", q=q_n),
            axis=X,
            op=OP.add,
        )
        lsr = sb.tile([P, mc_n], f32, name="lsr")
        nc.scalar.activation(lsr[:], srow[:], AF.Ln)

        # per-partition combine: F = 0.5*(sum lsr + sum lsc) - sum dterm
        s1 = sb.tile([P, 1], f32, name="s1")
        nc.vector.tensor_reduce(s1[:], lsr[:], axis=X, op=OP.add)
        s2 = sb.tile([P, 1], f32, name="s2")
        nc.vector.tensor_reduce(s2[:], lsc[:], axis=X, op=OP.add)
        s3 = sb.tile([P, 1], f32, name="s3")
        nc.vector.tensor_reduce(s3[:], dterm[:], axis=X, op=OP.add)
        tsum = sb.tile([P, 1], f32, name="tsum")
        nc.vector.tensor_add(tsum[:], s1[:], s2[:])
        fvec = sb.tile([P, 1], f32, name="fvec")
        nc.vector.scalar_tensor_tensor(
            out=fvec[:], in0=tsum[:], scalar=0.5, in1=s3[:],
            op0=OP.mult, op1=OP.subtract,
        )

        # partition sum -> scalar partial (scaled by 1/N), in a stolen
        # mm-tag PSUM slot
        loss_ps = ps.tile([P, GPW], f32, name="loss_ps", tag="mm", bufs=2)
        nc.tensor.matmul(loss_ps[0:1, 0:1], ones_f32[:], fvec[:], start=True,
                         stop=True)
        out_sb = sb.tile([1, 1], f32, name="out_sb")
        nc.scalar.mul(out_sb[:], loss_ps[0:1, 0:1], 1.0 / n_global)
        nc.gpsimd.dma_start(out_d[0:1, 0:1], out_sb[:])

    nc.compile()
    return nc


def make_in_maps(image_embeddings: np.ndarray, text_embeddings: np.ndarray):
    n_global, d = image_embeddings.shape
    m_loc = n_global // W
    cp_n = d // (2 * P)
    # host text normalization (f32): fold 16/||b_j|| into both text operands
    bn = np.sqrt((text_embeddings.astype(np.float32) ** 2).sum(axis=1,
                                                               keepdims=True))
    txt_n = text_embeddings * (BSCALE / np.maximum(bn, 1e-12))
    img8 = image_embeddings.astype(ml_dtypes.float8_e4m3fn)
    txtn8 = txt_n.astype(ml_dtypes.float8_e4m3fn)
    # fp8 transposed operands grouped for DoubleRow: [cp, 2, 128, cols]
    txtT8 = np.ascontiguousarray(
        np.ascontiguousarray(txtn8.T).reshape(cp_n, 2, P, n_global)
    )
    imgT8_full = np.ascontiguousarray(img8.T).reshape(cp_n, 2, P, n_global)
    return [
        {
            "img8": img8[k * m_loc:(k + 1) * m_loc],
            "txtn8": txtn8[k * m_loc:(k + 1) * m_loc],
            "txtT8": txtT8,
            "imgT8": np.ascontiguousarray(
                imgT8_full[:, :, :, k * m_loc:(k + 1) * m_loc]
            ),
        }
        for k in range(W)
    ]


def kernel(image_embeddings: np.ndarray, text_embeddings: np.ndarray) -> np.ndarray:
    from concourse.bass_utils import run_bass_kernel_spmd

    n_global, d = image_embeddings.shape
    key = (n_global, d)
    if key not in _CACHE:
        _CACHE[key] = build_bass(n_global, d)
    nc = _CACHE[key]

    in_maps = make_in_maps(
        np.asarray(image_embeddings, np.float32),
        np.asarray(text_embeddings, np.float32),
    )
    res = run_bass_kernel_spmd(nc, in_maps, core_ids=list(range(W)))
    total = sum(float(r["partial"][0, 0]) for r in res.results)
    return np.asarray(total, dtype=np.float32)
